# revision 18
# baseline (speedup 1.0000x reference)
"""Fused ReLU + 4x RMSNorm + 3x (matmul + residual-add) kernel for TRN2.

Reference computation (per token row t, hidden dim H=2048):
    x1 = relu(x); resid = x1
    for s in 0..2:
        y = rmsnorm(resid, g_s)                # norm over H
        resid = y @ W_s + resid
    out = rmsnorm(resid, g3)

Sharding: pure data-parallel over the token dim (32768 tokens -> 8 cores x
4096 tokens); W/g replicated per core, no collectives.

v7 design (v5 + engine rebalance + prefetch + decoupled seeds); measured
1669 us on 8xTRN2 vs the ~1630 us pure-matmul pacing floor of this part
(back-to-back N=512 matmuls measured at ~265 ns regardless of dtype or
LDWEIGHTS amortization, i.e. the PE streams at ~1.94 GHz effective):
  - The residual lives in PSUM: each 128-token tile owns a [128, 2048] fp32
    PSUM region (4 banks). DVE seeds it with relu(x)*WS, and each
    stage's matmuls accumulate y_s @ W_s' directly on top (start=False).
  - Matmul loop is k-outer/n-inner so each stationary y^T chunk is reused
    by 4 consecutive matmuls (one per 512-col PSUM slice).
  - Boundary chain per tile/stage: ScalarE Square (accum_out = row sum of
    squares), ScalarE Sqrt + DVE reciprocal (Rsqrt is blocked in bass),
    DVE tensor_scalar multiply PSUM -> bf16 y-hat (per-partition rs), xbar
    DMA transpose (SP queue) to the stationary layout. Out chain is one
    DVE scalar_tensor_tensor: (psum * rs) * g3 -> bf16, stored via
    Pool-queue SWDGE so SP stays clear for transposes/x-loads.
  - x tiles are prefetched one pair ahead (SP queue, own tag ring), and the
    stage-0 sumsq/y-hat chain runs from SBUF so it does not wait for the
    PSUM buffer to free.
  - All three W matrices are SBUF-resident in fp8e3 (12 MB total),
    host-prescaled by WS with g folded in; the kernel keeps
    resid' = WS * resid throughout. W is re-loaded once per For_i
    iteration so the bench accounts for its HBM traffic.
  - PSUM has_written warm-up runs once before the rep loop.
  - x arrives bf16, out leaves bf16 (host casts).
  - Rejected avenues (measured): k-outer LDW amortization and bf16-vs-fp8
    moving dtype do not change MM pacing; fp8e4 DoubleRow (2x PE rate)
    busts the 2e-2 error budget (3.3e-2 in numpy simulation -- e4m3 on
    either matmul operand alone already exceeds it); W0 double-buffering
    and Pool-engine PSUM seeding regressed or were neutral on HW.
"""

import sys

import numpy as np

try:
    import concourse.bass as bass  # noqa: F401
except ImportError:  # pragma: no cover
    sys.path.insert(0, "/opt/trn_rl_repo")

import concourse.bass as bass
import concourse.tile as tile
from concourse import bacc, mybir
from concourse.bass_utils import run_bass_kernel_spmd

import ml_dtypes

EPS = 1e-6
TOKENS = 32768
HIDDEN = 2048
N_CORES = 8
T_CORE = TOKENS // N_CORES  # 4096
TB = 512  # unused (kept for test.py compat)
F32 = mybir.dt.float32
BF16 = mybir.dt.bfloat16
FP8E3 = mybir.dt.float8e3
WS = 64.0


def build_program(t_core=T_CORE, hidden=HIDDEN, tb=TB, reps=1):
    """Build the per-core Bass program (SPMD: identical on all cores).
    reps>1 wraps the pipeline in a hardware For_i loop for slope timing."""
    nt_all = t_core // 128  # token tiles total (32)
    kc = hidden // 128      # contraction chunks (16)
    nb = hidden // 512      # output column blocks (4)
    assert t_core % 128 == 0 and hidden % 512 == 0

    nc = bacc.Bacc("TRN2", target_bir_lowering=False, debug=False)

    x_d = nc.dram_tensor("x", [t_core, hidden], BF16, kind="ExternalInput").ap()
    # W host-pretiled to [nb*128, kc*512]: row n*128+p holds W'[k*128+p,
    # n*512:(n+1)*512] for k=0..kc-1 contiguously.
    w_d = [
        nc.dram_tensor(
            f"W{i}", [nb * 128, kc * 512], FP8E3, kind="ExternalInput"
        ).ap()
        for i in range(3)
    ]
    g3_d = nc.dram_tensor("g3", [hidden], BF16, kind="ExternalInput").ap()
    out_d = nc.dram_tensor("out", [t_core, hidden], BF16, kind="ExternalOutput").ap()

    relu = mybir.ActivationFunctionType.Relu
    sqrt = mybir.ActivationFunctionType.Sqrt
    square = mybir.ActivationFunctionType.Square
    mult = mybir.AluOpType.mult

    with tile.TileContext(nc) as tc:
        with (
            tc.tile_pool(name="const", bufs=1) as const_pool,
            tc.tile_pool(name="w", bufs=1) as w_pool,
            tc.tile_pool(name="yhat", bufs=6) as yhat_pool,
            tc.tile_pool(name="yT", bufs=4) as yt_pool,
            tc.tile_pool(name="small", bufs=16) as small_pool,
            tc.tile_pool(name="psum", bufs=2, space="PSUM") as psum_pool,
        ):
            eps_t = const_pool.tile([128, 1], F32)
            nc.vector.memset(eps_t, EPS * WS * WS)
            eps0_t = const_pool.tile([128, 1], F32)
            nc.vector.memset(eps0_t, EPS)

            def bcast(ap):
                return bass.AP(
                    tensor=ap.tensor, offset=ap.offset, ap=[[0, 128]] + list(ap.ap)
                )

            g3t = const_pool.tile([128, hidden], BF16, tag="g3")
            nc.gpsimd.dma_start(out=g3t, in_=bcast(g3_d))
            sq_scr = const_pool.tile([128, hidden], BF16, tag="sqscr")
            zt = const_pool.tile([128, 128], BF16, tag="zt")
            nc.vector.memset(zt, 0.0)

            w_re = [
                w.rearrange("(n p) (k c) -> p n k c", p=128, k=kc) for w in w_d
            ]

            # PSUM warm-up, ONCE before the rep loop: the seeded-accumulate
            # pattern (ScalarE write + matmul start=False) only accumulates
            # if the bank's has_written state is set; on a cold core the
            # first start=False group lazily zeroes instead, dropping the
            # seed. One start=True dummy matmul per 512-col region of both
            # PSUM buffers sets the bits; within the loop nothing clears
            # them (all real matmuls use start=False).
            for j in range(2):
                pw = psum_pool.tile([128, hidden], F32, tag="pr", name=f"warm{j}")
                for n in range(nb):
                    nc.tensor.matmul(
                        pw[:, n * 512 : (n + 1) * 512],
                        zt,
                        g3t[:, :512],
                        start=True,
                        stop=True,
                    )

            def body():
                pr_of = {}
                ss_of = {}
                xt_of = {}

                def xload(m):
                    """Prefetch a token tile of x (issued a pair ahead)."""
                    xt = yhat_pool.tile(
                        [128, hidden], BF16, tag="xt", bufs=4, name=f"x{m}"
                    )
                    nc.sync.dma_start(
                        out=xt, in_=x_d[m * 128 : (m + 1) * 128, :]
                    )
                    xt_of[m] = xt

                # First pair's x tiles BEFORE the W loads: at a rep boundary
                # the W loads' buffer-WAR waits would otherwise head-of-line
                # block the seed chain's x loads in the in-order SP queue.
                xload(0)
                xload(1)

                # W loads inside the loop body so steady-state timing
                # includes their HBM traffic (they overlap compute).
                wt = []
                for s in range(3):
                    w_s = w_pool.tile(
                        [128, nb, kc, 512], FP8E3, tag=f"w{s}", name=f"w{s}"
                    )
                    nc.sync.dma_start(out=w_s, in_=w_re[s])
                    wt.append(w_s)

                def rs_chain(m, s, eps):
                    """sqrt (ScalarE) + reciprocal (DVE) from the squares."""
                    rs = small_pool.tile(
                        [128, 1], F32, tag="rs", name=f"rs{m}_{s}"
                    )
                    nc.scalar.activation(
                        out=rs, in_=ss_of[m], func=sqrt, bias=eps[:, :],
                        scale=1.0 / hidden,
                    )
                    nc.vector.reciprocal(rs, rs)
                    return rs

                def new_ss(m, s):
                    ss = small_pool.tile(
                        [128, 1], F32, tag="ss", name=f"ss{m}_{s}"
                    )
                    ss_of[m] = ss
                    return ss

                def seed_chain(m):
                    """relu (SBUF), row sumsq, stage-0 y-hat, PSUM seed.

                    The sumsq/y-hat chain runs entirely from SBUF so it does
                    not wait for the PSUM buffer to free; only the WS*relu(x)
                    seed write (DVE, off the critical path) needs PSUM."""
                    xt = xt_of.pop(m)
                    r = yhat_pool.tile(
                        [128, hidden], BF16, tag="yh", name=f"r{m}"
                    )
                    nc.scalar.activation(out=r, in_=xt, func=relu)
                    nc.scalar.activation(
                        out=sq_scr, in_=r, func=square,
                        accum_out=new_ss(m, 0)[:, :],
                    )
                    rs = rs_chain(m, 0, eps0_t)
                    yh = yhat_pool.tile(
                        [128, hidden], BF16, tag="yh", name=f"yh{m}_0"
                    )
                    nc.vector.tensor_scalar_mul(yh, r, rs[:, :])
                    yt = yt_pool.tile(
                        [128, kc, 128], BF16, tag="yT", name=f"yt{m}_0"
                    )
                    nc.sync.dma_start_transpose(yt, yh)
                    pr = psum_pool.tile(
                        [128, hidden], F32, tag="pr", name=f"pr{m}"
                    )
                    pr_of[m] = pr
                    nc.vector.tensor_scalar_mul(pr, r, float(WS))
                    return yt

                def cast_t(m, s):
                    """rs, DVE scaled copy PSUM->bf16 y-hat, xbar transpose."""
                    rs = rs_chain(m, s, eps_t)
                    yh = yhat_pool.tile(
                        [128, hidden], BF16, tag="yh", name=f"yh{m}_{s}"
                    )
                    nc.vector.tensor_scalar_mul(yh, pr_of[m], rs[:, :])
                    yt = yt_pool.tile(
                        [128, kc, 128], BF16, tag="yT", name=f"yt{m}_{s}"
                    )
                    nc.sync.dma_start_transpose(yt, yh)
                    return yt

                def mm(m, s, yt):
                    # k-outer / n-inner: the stationary yt[:, k, :] is
                    # reused by 4 consecutive matmuls (one per PSUM slice).
                    pr = pr_of[m]
                    for k in range(kc):
                        for n in range(nb):
                            nc.tensor.matmul(
                                pr[:, n * 512 : (n + 1) * 512],
                                yt[:, k, :],
                                wt[s][:, n, k, :],
                                start=False,
                                stop=(k == kc - 1),
                                skip_group_check=True,
                            )

                def sq_then_cast(m, s):
                    """After stage s-1's matmuls: row sumsq on ScalarE."""
                    nc.scalar.activation(
                        out=sq_scr, in_=pr_of[m], func=square,
                        accum_out=new_ss(m, s)[:, :],
                    )

                def out_chain(m):
                    rs = rs_chain(m, 3, eps_t)
                    y3 = yhat_pool.tile(
                        [128, hidden], BF16, tag="yh", name=f"y3_{m}"
                    )
                    # y3 = (psum * rs) * g3 in one DVE op
                    nc.vector.scalar_tensor_tensor(
                        out=y3, in0=pr_of[m], scalar=rs[:, :], in1=g3t,
                        op0=mult, op1=mult,
                    )
                    nc.gpsimd.dma_start(
                        out=out_d[m * 128 : (m + 1) * 128, :], in_=y3
                    )

                # Pair-lockstep software pipeline: tiles (A, B) march
                # through stages together, so each tile's boundary chain
                # hides under the other tile's matmul block. The previous
                # pair's out-chains and this pair's seeds are interleaved
                # so they run in the shadow of the trailing matmul blocks.
                prev = None
                for a in range(0, nt_all, 2):
                    b = a + 1
                    if a + 2 < nt_all:
                        xload(a + 2)
                        xload(b + 2)
                    if prev is None:
                        ytA = seed_chain(a)
                        ytB = seed_chain(b)
                        mm(a, 0, ytA)
                        mm(b, 0, ytB)
                    else:
                        pa, pb = prev
                        sq_then_cast(pa, 3)
                        out_chain(pa)
                        ytA = seed_chain(a)
                        mm(a, 0, ytA)
                        sq_then_cast(pb, 3)
                        out_chain(pb)
                        ytB = seed_chain(b)
                        mm(b, 0, ytB)
                    for s in (1, 2):
                        sq_then_cast(a, s)
                        ytA = cast_t(a, s)
                        mm(a, s, ytA)
                        sq_then_cast(b, s)
                        ytB = cast_t(b, s)
                        mm(b, s, ytB)
                    prev = (a, b)
                pa, pb = prev
                sq_then_cast(pa, 3)
                out_chain(pa)
                sq_then_cast(pb, 3)
                out_chain(pb)

            if reps == 1:
                body()
            elif reps < 0:  # unrolled (sim analysis only)
                for _ in range(-reps):
                    body()
            else:
                with tc.For_i(0, reps, 1):
                    body()

    nc.compile()
    return nc


_CACHE = {}


def _get_program(key=(T_CORE, HIDDEN, TB)):  # noqa: B008
    if key not in _CACHE:
        _CACHE[key] = build_program(*key)
    return _CACHE[key]


def make_in_maps(inputs):
    """Host-side prep: fold g into W, scale by WS, cast, pre-tile, shard."""
    x = np.asarray(inputs["x"], dtype=np.float32).astype(ml_dtypes.bfloat16)
    kc, nb = HIDDEN // 128, HIDDEN // 512
    ws = []
    for i in range(3):
        w = np.asarray(inputs[f"W{i}"], dtype=np.float32)
        g = np.asarray(inputs[f"g{i}"], dtype=np.float32)
        w8 = (WS * g[:, None] * w).astype(ml_dtypes.float8_e3m4)
        w8 = w8.reshape(kc, 128, nb, 512).transpose(2, 1, 0, 3)
        ws.append(np.ascontiguousarray(w8.reshape(nb * 128, kc * 512)))
    g3 = np.asarray(inputs["g3"], dtype=np.float32).astype(ml_dtypes.bfloat16)

    in_maps = []
    for c in range(N_CORES):
        im = {"x": np.ascontiguousarray(x[c * T_CORE : (c + 1) * T_CORE])}
        for i in range(3):
            im[f"W{i}"] = ws[i]
        im["g3"] = g3
        in_maps.append(im)
    return in_maps


def run(inputs, trace=False):
    """Run on 8 NeuronCores. Returns (out, BassKernelResults)."""
    nc = _get_program()
    in_maps = make_in_maps(inputs)
    res = run_bass_kernel_spmd(nc, in_maps, list(range(N_CORES)), trace=trace)
    out = np.concatenate(
        [res.results[c]["out"].astype(np.float32) for c in range(N_CORES)], axis=0
    )
    return out, res


def kernel(**inputs) -> np.ndarray:
    out, _ = run(inputs, trace=False)
    return out


# revision 19
# speedup vs baseline: 1.0005x; 1.0005x over previous
"""Fused ReLU + 4x RMSNorm + 3x (matmul + residual-add) kernel for TRN2.

Reference computation (per token row t, hidden dim H=2048):
    x1 = relu(x); resid = x1
    for s in 0..2:
        y = rmsnorm(resid, g_s)                # norm over H
        resid = y @ W_s + resid
    out = rmsnorm(resid, g3)

Sharding: pure data-parallel over the token dim (32768 tokens -> 8 cores x
4096 tokens); W/g replicated per core, no collectives.

v7 design (v5 + engine rebalance + prefetch + decoupled seeds); measured
1669 us on 8xTRN2 vs the ~1630 us pure-matmul pacing floor of this part
(back-to-back N=512 matmuls measured at ~265 ns regardless of dtype or
LDWEIGHTS amortization, i.e. the PE streams at ~1.94 GHz effective):
  - The residual lives in PSUM: each 128-token tile owns a [128, 2048] fp32
    PSUM region (4 banks). DVE seeds it with relu(x)*WS, and each
    stage's matmuls accumulate y_s @ W_s' directly on top (start=False).
  - Matmul loop is k-outer/n-inner so each stationary y^T chunk is reused
    by 4 consecutive matmuls (one per 512-col PSUM slice).
  - Boundary chain per tile/stage: ScalarE Square (accum_out = row sum of
    squares), ScalarE Sqrt + DVE reciprocal (Rsqrt is blocked in bass),
    DVE tensor_scalar multiply PSUM -> bf16 y-hat (per-partition rs), xbar
    DMA transpose (SP queue) to the stationary layout. Out chain is one
    DVE scalar_tensor_tensor: (psum * rs) * g3 -> bf16, stored via
    Pool-queue SWDGE so SP stays clear for transposes/x-loads.
  - x tiles are prefetched one pair ahead (SP queue, own tag ring), and the
    stage-0 sumsq/y-hat chain runs from SBUF so it does not wait for the
    PSUM buffer to free.
  - All three W matrices are SBUF-resident in fp8e3 (12 MB total),
    host-prescaled by WS with g folded in; the kernel keeps
    resid' = WS * resid throughout. W is re-loaded once per For_i
    iteration so the bench accounts for its HBM traffic.
  - PSUM has_written warm-up runs once before the rep loop.
  - x arrives bf16, out leaves bf16 (host casts).
  - Rejected avenues (measured): k-outer LDW amortization and bf16-vs-fp8
    moving dtype do not change MM pacing; fp8e4 DoubleRow (2x PE rate)
    busts the 2e-2 error budget (3.3e-2 in numpy simulation -- e4m3 on
    either matmul operand alone already exceeds it); W0 double-buffering
    and Pool-engine PSUM seeding regressed or were neutral on HW.
"""

import sys

import numpy as np

try:
    import concourse.bass as bass  # noqa: F401
except ImportError:  # pragma: no cover
    sys.path.insert(0, "/opt/trn_rl_repo")

import concourse.bass as bass
import concourse.tile as tile
from concourse import bacc, mybir
from concourse.bass_utils import run_bass_kernel_spmd

import ml_dtypes

EPS = 1e-6
TOKENS = 32768
HIDDEN = 2048
N_CORES = 8
T_CORE = TOKENS // N_CORES  # 4096
TB = 512  # unused (kept for test.py compat)
F32 = mybir.dt.float32
BF16 = mybir.dt.bfloat16
FP8E3 = mybir.dt.float8e3
WS = 64.0


def build_program(t_core=T_CORE, hidden=HIDDEN, tb=TB, reps=1):
    """Build the per-core Bass program (SPMD: identical on all cores).
    reps>1 wraps the pipeline in a hardware For_i loop for slope timing."""
    nt_all = t_core // 128  # token tiles total (32)
    kc = hidden // 128      # contraction chunks (16)
    nb = hidden // 512      # output column blocks (4)
    assert t_core % 128 == 0 and hidden % 512 == 0

    nc = bacc.Bacc("TRN2", target_bir_lowering=False, debug=False)

    x_d = nc.dram_tensor("x", [t_core, hidden], BF16, kind="ExternalInput").ap()
    # W host-pretiled to [nb*128, kc*512]: row n*128+p holds W'[k*128+p,
    # n*512:(n+1)*512] for k=0..kc-1 contiguously.
    w_d = [
        nc.dram_tensor(
            f"W{i}", [nb * 128, kc * 512], FP8E3, kind="ExternalInput"
        ).ap()
        for i in range(3)
    ]
    g3_d = nc.dram_tensor("g3", [hidden], BF16, kind="ExternalInput").ap()
    out_d = nc.dram_tensor("out", [t_core, hidden], BF16, kind="ExternalOutput").ap()

    relu = mybir.ActivationFunctionType.Relu
    sqrt = mybir.ActivationFunctionType.Sqrt
    square = mybir.ActivationFunctionType.Square
    mult = mybir.AluOpType.mult

    with tile.TileContext(nc) as tc:
        with (
            tc.tile_pool(name="const", bufs=1) as const_pool,
            tc.tile_pool(name="w0", bufs=1) as w0_pool,
            tc.tile_pool(name="w1", bufs=1) as w1_pool,
            tc.tile_pool(name="w2", bufs=1) as w2_pool,
            tc.tile_pool(name="yhat", bufs=6) as yhat_pool,
            tc.tile_pool(name="yT", bufs=4) as yt_pool,
            tc.tile_pool(name="small", bufs=16) as small_pool,
            tc.tile_pool(name="psum", bufs=2, space="PSUM") as psum_pool,
        ):
            eps_t = const_pool.tile([128, 1], F32)
            nc.vector.memset(eps_t, EPS * WS * WS)
            eps0_t = const_pool.tile([128, 1], F32)
            nc.vector.memset(eps0_t, EPS)

            def bcast(ap):
                return bass.AP(
                    tensor=ap.tensor, offset=ap.offset, ap=[[0, 128]] + list(ap.ap)
                )

            g3t = const_pool.tile([128, hidden], BF16, tag="g3")
            nc.gpsimd.dma_start(out=g3t, in_=bcast(g3_d))
            sq_scr = const_pool.tile([128, hidden], BF16, tag="sqscr")
            zt = const_pool.tile([128, 128], BF16, tag="zt")
            nc.vector.memset(zt, 0.0)

            w_re = [
                w.rearrange("(n p) (k c) -> p n k c", p=128, k=kc) for w in w_d
            ]

            # PSUM warm-up, ONCE before the rep loop: the seeded-accumulate
            # pattern (ScalarE write + matmul start=False) only accumulates
            # if the bank's has_written state is set; on a cold core the
            # first start=False group lazily zeroes instead, dropping the
            # seed. One start=True dummy matmul per 512-col region of both
            # PSUM buffers sets the bits; within the loop nothing clears
            # them (all real matmuls use start=False).
            for j in range(2):
                pw = psum_pool.tile([128, hidden], F32, tag="pr", name=f"warm{j}")
                for n in range(nb):
                    nc.tensor.matmul(
                        pw[:, n * 512 : (n + 1) * 512],
                        zt,
                        g3t[:, :512],
                        start=True,
                        stop=True,
                    )

            def body():
                pr_of = {}
                ss_of = {}
                xt_of = {}

                def xload(m):
                    """Prefetch a token tile of x (issued a pair ahead)."""
                    xt = yhat_pool.tile(
                        [128, hidden], BF16, tag="xt", bufs=4, name=f"x{m}"
                    )
                    nc.sync.dma_start(
                        out=xt, in_=x_d[m * 128 : (m + 1) * 128, :]
                    )
                    xt_of[m] = xt

                # First pair's x tiles BEFORE the W loads: at a rep boundary
                # the W loads' buffer-WAR waits would otherwise head-of-line
                # block the seed chain's x loads in the in-order SP queue.
                xload(0)
                xload(1)

                # W loads inside the loop body so steady-state timing
                # includes their HBM traffic (they overlap compute).
                wt = []
                for s, wp in enumerate((w0_pool, w1_pool, w2_pool)):
                    w_s = wp.tile(
                        [128, nb, kc, 512], FP8E3, tag=f"w{s}", name=f"w{s}"
                    )
                    nc.sync.dma_start(out=w_s, in_=w_re[s])
                    wt.append(w_s)

                def rs_chain(m, s, eps):
                    """sqrt (ScalarE) + reciprocal (DVE) from the squares."""
                    rs = small_pool.tile(
                        [128, 1], F32, tag="rs", name=f"rs{m}_{s}"
                    )
                    nc.scalar.activation(
                        out=rs, in_=ss_of[m], func=sqrt, bias=eps[:, :],
                        scale=1.0 / hidden,
                    )
                    nc.vector.reciprocal(rs, rs)
                    return rs

                def new_ss(m, s):
                    ss = small_pool.tile(
                        [128, 1], F32, tag="ss", name=f"ss{m}_{s}"
                    )
                    ss_of[m] = ss
                    return ss

                def seed_chain(m):
                    """relu (SBUF), row sumsq, stage-0 y-hat, PSUM seed.

                    The sumsq/y-hat chain runs entirely from SBUF so it does
                    not wait for the PSUM buffer to free; only the WS*relu(x)
                    seed write (DVE, off the critical path) needs PSUM."""
                    xt = xt_of.pop(m)
                    r = yhat_pool.tile(
                        [128, hidden], BF16, tag="yh", name=f"r{m}"
                    )
                    nc.scalar.activation(out=r, in_=xt, func=relu)
                    nc.scalar.activation(
                        out=sq_scr, in_=r, func=square,
                        accum_out=new_ss(m, 0)[:, :],
                    )
                    rs = rs_chain(m, 0, eps0_t)
                    yh = yhat_pool.tile(
                        [128, hidden], BF16, tag="yh", name=f"yh{m}_0"
                    )
                    nc.vector.tensor_scalar_mul(yh, r, rs[:, :])
                    yt = yt_pool.tile(
                        [128, kc, 128], BF16, tag="yT", name=f"yt{m}_0"
                    )
                    nc.sync.dma_start_transpose(yt, yh)
                    pr = psum_pool.tile(
                        [128, hidden], F32, tag="pr", name=f"pr{m}"
                    )
                    pr_of[m] = pr
                    nc.vector.tensor_scalar_mul(pr, r, float(WS))
                    return yt

                def cast_t(m, s):
                    """rs, DVE scaled copy PSUM->bf16 y-hat, xbar transpose."""
                    rs = rs_chain(m, s, eps_t)
                    yh = yhat_pool.tile(
                        [128, hidden], BF16, tag="yh", name=f"yh{m}_{s}"
                    )
                    nc.vector.tensor_scalar_mul(yh, pr_of[m], rs[:, :])
                    yt = yt_pool.tile(
                        [128, kc, 128], BF16, tag="yT", name=f"yt{m}_{s}"
                    )
                    nc.sync.dma_start_transpose(yt, yh)
                    return yt

                def mm(m, s, yt):
                    # k-outer / n-inner: the stationary yt[:, k, :] is
                    # reused by 4 consecutive matmuls (one per PSUM slice).
                    pr = pr_of[m]
                    for k in range(kc):
                        for n in range(nb):
                            nc.tensor.matmul(
                                pr[:, n * 512 : (n + 1) * 512],
                                yt[:, k, :],
                                wt[s][:, n, k, :],
                                start=False,
                                stop=(k == kc - 1),
                                skip_group_check=True,
                            )

                def sq_then_cast(m, s):
                    """After stage s-1's matmuls: row sumsq on ScalarE."""
                    nc.scalar.activation(
                        out=sq_scr, in_=pr_of[m], func=square,
                        accum_out=new_ss(m, s)[:, :],
                    )

                def out_chain(m):
                    rs = rs_chain(m, 3, eps_t)
                    y3 = yhat_pool.tile(
                        [128, hidden], BF16, tag="yh", name=f"y3_{m}"
                    )
                    # y3 = (psum * rs) * g3 in one DVE op
                    nc.vector.scalar_tensor_tensor(
                        out=y3, in0=pr_of[m], scalar=rs[:, :], in1=g3t,
                        op0=mult, op1=mult,
                    )
                    nc.gpsimd.dma_start(
                        out=out_d[m * 128 : (m + 1) * 128, :], in_=y3
                    )

                # Pair-lockstep software pipeline: tiles (A, B) march
                # through stages together, so each tile's boundary chain
                # hides under the other tile's matmul block. The previous
                # pair's out-chains and this pair's seeds are interleaved
                # so they run in the shadow of the trailing matmul blocks.
                prev = None
                for a in range(0, nt_all, 2):
                    b = a + 1
                    if a + 2 < nt_all:
                        xload(a + 2)
                        xload(b + 2)
                    if prev is None:
                        ytA = seed_chain(a)
                        ytB = seed_chain(b)
                        mm(a, 0, ytA)
                        mm(b, 0, ytB)
                    else:
                        pa, pb = prev
                        sq_then_cast(pa, 3)
                        out_chain(pa)
                        ytA = seed_chain(a)
                        mm(a, 0, ytA)
                        sq_then_cast(pb, 3)
                        out_chain(pb)
                        ytB = seed_chain(b)
                        mm(b, 0, ytB)
                    for s in (1, 2):
                        sq_then_cast(a, s)
                        ytA = cast_t(a, s)
                        mm(a, s, ytA)
                        sq_then_cast(b, s)
                        ytB = cast_t(b, s)
                        mm(b, s, ytB)
                    prev = (a, b)
                pa, pb = prev
                sq_then_cast(pa, 3)
                out_chain(pa)
                sq_then_cast(pb, 3)
                out_chain(pb)

            if reps == 1:
                body()
            elif reps < 0:  # unrolled (sim analysis only)
                for _ in range(-reps):
                    body()
            else:
                with tc.For_i(0, reps, 1):
                    body()

    nc.compile()
    return nc


_CACHE = {}


def _get_program(key=(T_CORE, HIDDEN, TB)):  # noqa: B008
    if key not in _CACHE:
        _CACHE[key] = build_program(*key)
    return _CACHE[key]


def make_in_maps(inputs):
    """Host-side prep: fold g into W, scale by WS, cast, pre-tile, shard."""
    x = np.asarray(inputs["x"], dtype=np.float32).astype(ml_dtypes.bfloat16)
    kc, nb = HIDDEN // 128, HIDDEN // 512
    ws = []
    for i in range(3):
        w = np.asarray(inputs[f"W{i}"], dtype=np.float32)
        g = np.asarray(inputs[f"g{i}"], dtype=np.float32)
        w8 = (WS * g[:, None] * w).astype(ml_dtypes.float8_e3m4)
        w8 = w8.reshape(kc, 128, nb, 512).transpose(2, 1, 0, 3)
        ws.append(np.ascontiguousarray(w8.reshape(nb * 128, kc * 512)))
    g3 = np.asarray(inputs["g3"], dtype=np.float32).astype(ml_dtypes.bfloat16)

    in_maps = []
    for c in range(N_CORES):
        im = {"x": np.ascontiguousarray(x[c * T_CORE : (c + 1) * T_CORE])}
        for i in range(3):
            im[f"W{i}"] = ws[i]
        im["g3"] = g3
        in_maps.append(im)
    return in_maps


def run(inputs, trace=False):
    """Run on 8 NeuronCores. Returns (out, BassKernelResults)."""
    nc = _get_program()
    in_maps = make_in_maps(inputs)
    res = run_bass_kernel_spmd(nc, in_maps, list(range(N_CORES)), trace=trace)
    out = np.concatenate(
        [res.results[c]["out"].astype(np.float32) for c in range(N_CORES)], axis=0
    )
    return out, res


def kernel(**inputs) -> np.ndarray:
    out, _ = run(inputs, trace=False)
    return out


# revision 20
# speedup vs baseline: 1.0029x; 1.0025x over previous
"""Fused ReLU + 4x RMSNorm + 3x (matmul + residual-add) kernel for TRN2.

Reference computation (per token row t, hidden dim H=2048):
    x1 = relu(x); resid = x1
    for s in 0..2:
        y = rmsnorm(resid, g_s)                # norm over H
        resid = y @ W_s + resid
    out = rmsnorm(resid, g3)

Sharding: pure data-parallel over the token dim (32768 tokens -> 8 cores x
4096 tokens); W/g replicated per core, no collectives.

v9 design (v5 + engine rebalance + prefetch + decoupled seeds + precise
W-load deps); measured ~1663 us on 8xTRN2 vs the ~1630 us pure-matmul
pacing floor of this part (back-to-back N=512 matmuls measured at ~265 ns
regardless of dtype or LDWEIGHTS amortization, i.e. the PE streams at
~1.94 GHz effective; per-MM time is N-linear with zero fixed overhead):
  - The residual lives in PSUM: each 128-token tile owns a [128, 2048] fp32
    PSUM region (4 banks). DVE seeds it with relu(x)*WS, and each
    stage's matmuls accumulate y_s @ W_s' directly on top (start=False).
  - Matmul loop is k-outer/n-inner so each stationary y^T chunk is reused
    by 4 consecutive matmuls (one per 512-col PSUM slice).
  - Boundary chain per tile/stage: ScalarE Square (accum_out = row sum of
    squares), ScalarE Sqrt + DVE reciprocal (Rsqrt is blocked in bass),
    DVE tensor_scalar multiply PSUM -> bf16 y-hat (per-partition rs), xbar
    DMA transpose (SP queue) to the stationary layout. Out chain is one
    DVE scalar_tensor_tensor: (psum * rs) * g3 -> bf16, stored via
    Pool-queue SWDGE so SP stays clear for transposes/x-loads.
  - x tiles are prefetched one pair ahead (SP queue, own tag ring), and the
    stage-0 sumsq/y-hat chain runs from SBUF so it does not wait for the
    PSUM buffer to free.
  - All three W matrices are SBUF-resident in fp8e3 (12 MB total),
    host-prescaled by WS with g folded in; the kernel keeps
    resid' = WS * resid throughout. W is re-loaded once per For_i
    iteration so the bench accounts for its HBM traffic.
  - PSUM has_written warm-up runs once before the rep loop.
  - x arrives bf16, out leaves bf16 (host casts).
  - Each W lives in its own tile pool so the per-rep reload's WAR wait is
    on that stage's true last reader (PE sem 5896/6024/6152) instead of
    the whole previous rep; first-pair x loads are issued ahead of the W
    loads so the W waits cannot head-of-line block the seed chain in the
    in-order SP queue.
  - Rejected avenues (measured): k-outer LDW amortization and bf16-vs-fp8
    moving dtype do not change MM pacing; fp8e4 DoubleRow (2x PE rate)
    busts the 2e-2 error budget (3.3e-2 in numpy simulation -- e4m3 on
    either matmul operand alone already exceeds it); W0 double-buffering
    regressed on HW (+42 us, likely SBUF placement); Pool-engine PSUM
    seeding and yT pool growth were neutral in sim.
"""

import sys

import numpy as np

try:
    import concourse.bass as bass  # noqa: F401
except ImportError:  # pragma: no cover
    sys.path.insert(0, "/opt/trn_rl_repo")

import concourse.bass as bass
import concourse.tile as tile
from concourse import bacc, mybir
from concourse.bass_utils import run_bass_kernel_spmd

import ml_dtypes

EPS = 1e-6
TOKENS = 32768
HIDDEN = 2048
N_CORES = 8
T_CORE = TOKENS // N_CORES  # 4096
TB = 512  # unused (kept for test.py compat)
F32 = mybir.dt.float32
BF16 = mybir.dt.bfloat16
FP8E3 = mybir.dt.float8e3
WS = 64.0


def build_program(t_core=T_CORE, hidden=HIDDEN, tb=TB, reps=1):
    """Build the per-core Bass program (SPMD: identical on all cores).
    reps>1 wraps the pipeline in a hardware For_i loop for slope timing."""
    nt_all = t_core // 128  # token tiles total (32)
    kc = hidden // 128      # contraction chunks (16)
    nb = hidden // 512      # output column blocks (4)
    assert t_core % 128 == 0 and hidden % 512 == 0

    nc = bacc.Bacc("TRN2", target_bir_lowering=False, debug=False)

    x_d = nc.dram_tensor("x", [t_core, hidden], BF16, kind="ExternalInput").ap()
    # W host-pretiled to [nb*128, kc*512]: row n*128+p holds W'[k*128+p,
    # n*512:(n+1)*512] for k=0..kc-1 contiguously.
    w_d = [
        nc.dram_tensor(
            f"W{i}", [nb * 128, kc * 512], FP8E3, kind="ExternalInput"
        ).ap()
        for i in range(3)
    ]
    g3_d = nc.dram_tensor("g3", [hidden], BF16, kind="ExternalInput").ap()
    out_d = nc.dram_tensor("out", [t_core, hidden], BF16, kind="ExternalOutput").ap()

    relu = mybir.ActivationFunctionType.Relu
    sqrt = mybir.ActivationFunctionType.Sqrt
    square = mybir.ActivationFunctionType.Square
    mult = mybir.AluOpType.mult

    with tile.TileContext(nc) as tc:
        with (
            tc.tile_pool(name="const", bufs=1) as const_pool,
            tc.tile_pool(name="w0", bufs=1) as w0_pool,
            tc.tile_pool(name="w1", bufs=1) as w1_pool,
            tc.tile_pool(name="w2", bufs=1) as w2_pool,
            tc.tile_pool(name="yhat", bufs=6) as yhat_pool,
            tc.tile_pool(name="yT", bufs=4) as yt_pool,
            tc.tile_pool(name="small", bufs=16) as small_pool,
            tc.tile_pool(name="psum", bufs=2, space="PSUM") as psum_pool,
        ):
            eps_t = const_pool.tile([128, 1], F32)
            nc.vector.memset(eps_t, EPS * WS * WS)
            eps0_t = const_pool.tile([128, 1], F32)
            nc.vector.memset(eps0_t, EPS)

            def bcast(ap):
                return bass.AP(
                    tensor=ap.tensor, offset=ap.offset, ap=[[0, 128]] + list(ap.ap)
                )

            g3t = const_pool.tile([128, hidden], BF16, tag="g3")
            nc.gpsimd.dma_start(out=g3t, in_=bcast(g3_d))
            sq_scr = const_pool.tile([128, hidden], BF16, tag="sqscr")
            zt = const_pool.tile([128, 128], BF16, tag="zt")
            nc.vector.memset(zt, 0.0)

            w_re = [
                w.rearrange("(n p) (k c) -> p n k c", p=128, k=kc) for w in w_d
            ]

            # PSUM warm-up, ONCE before the rep loop: the seeded-accumulate
            # pattern (ScalarE write + matmul start=False) only accumulates
            # if the bank's has_written state is set; on a cold core the
            # first start=False group lazily zeroes instead, dropping the
            # seed. One start=True dummy matmul per 512-col region of both
            # PSUM buffers sets the bits; within the loop nothing clears
            # them (all real matmuls use start=False).
            for j in range(2):
                pw = psum_pool.tile([128, hidden], F32, tag="pr", name=f"warm{j}")
                for n in range(nb):
                    nc.tensor.matmul(
                        pw[:, n * 512 : (n + 1) * 512],
                        zt,
                        g3t[:, :512],
                        start=True,
                        stop=True,
                    )

            def body():
                pr_of = {}
                ss_of = {}
                xt_of = {}

                def xload(m):
                    """Prefetch a token tile of x (issued a pair ahead)."""
                    xt = yhat_pool.tile(
                        [128, hidden], BF16, tag="xt", bufs=4, name=f"x{m}"
                    )
                    nc.sync.dma_start(
                        out=xt, in_=x_d[m * 128 : (m + 1) * 128, :]
                    )
                    xt_of[m] = xt

                # First pair's x tiles BEFORE the W loads: at a rep boundary
                # the W loads' buffer-WAR waits would otherwise head-of-line
                # block the seed chain's x loads in the in-order SP queue.
                xload(0)
                xload(1)

                # W loads inside the loop body so steady-state timing
                # includes their HBM traffic (they overlap compute).
                wt = []
                for s, wp in enumerate((w0_pool, w1_pool, w2_pool)):
                    w_s = wp.tile(
                        [128, nb, kc, 512], FP8E3, tag=f"w{s}", name=f"w{s}"
                    )
                    nc.sync.dma_start(out=w_s, in_=w_re[s])
                    wt.append(w_s)

                def rs_chain(m, s, eps):
                    """sqrt (ScalarE) + reciprocal (DVE) from the squares."""
                    rs = small_pool.tile(
                        [128, 1], F32, tag="rs", name=f"rs{m}_{s}"
                    )
                    nc.scalar.activation(
                        out=rs, in_=ss_of[m], func=sqrt, bias=eps[:, :],
                        scale=1.0 / hidden,
                    )
                    nc.vector.reciprocal(rs, rs)
                    return rs

                def new_ss(m, s):
                    ss = small_pool.tile(
                        [128, 1], F32, tag="ss", name=f"ss{m}_{s}"
                    )
                    ss_of[m] = ss
                    return ss

                def seed_chain(m):
                    """relu (SBUF), row sumsq, stage-0 y-hat, PSUM seed.

                    The sumsq/y-hat chain runs entirely from SBUF so it does
                    not wait for the PSUM buffer to free; only the WS*relu(x)
                    seed write (DVE, off the critical path) needs PSUM."""
                    xt = xt_of.pop(m)
                    r = yhat_pool.tile(
                        [128, hidden], BF16, tag="yh", name=f"r{m}"
                    )
                    nc.scalar.activation(out=r, in_=xt, func=relu)
                    nc.scalar.activation(
                        out=sq_scr, in_=r, func=square,
                        accum_out=new_ss(m, 0)[:, :],
                    )
                    rs = rs_chain(m, 0, eps0_t)
                    yh = yhat_pool.tile(
                        [128, hidden], BF16, tag="yh", name=f"yh{m}_0"
                    )
                    nc.vector.tensor_scalar_mul(yh, r, rs[:, :])
                    yt = yt_pool.tile(
                        [128, kc, 128], BF16, tag="yT", name=f"yt{m}_0"
                    )
                    nc.sync.dma_start_transpose(yt, yh)
                    pr = psum_pool.tile(
                        [128, hidden], F32, tag="pr", name=f"pr{m}"
                    )
                    pr_of[m] = pr
                    nc.vector.tensor_scalar_mul(pr, r, float(WS))
                    return yt

                def cast_t(m, s):
                    """rs, DVE scaled copy PSUM->bf16 y-hat, xbar transpose."""
                    rs = rs_chain(m, s, eps_t)
                    yh = yhat_pool.tile(
                        [128, hidden], BF16, tag="yh", name=f"yh{m}_{s}"
                    )
                    nc.vector.tensor_scalar_mul(yh, pr_of[m], rs[:, :])
                    yt = yt_pool.tile(
                        [128, kc, 128], BF16, tag="yT", name=f"yt{m}_{s}"
                    )
                    nc.sync.dma_start_transpose(yt, yh)
                    return yt

                def mm(m, s, yt):
                    # k-outer / n-inner: the stationary yt[:, k, :] is
                    # reused by 4 consecutive matmuls (one per PSUM slice).
                    pr = pr_of[m]
                    for k in range(kc):
                        for n in range(nb):
                            nc.tensor.matmul(
                                pr[:, n * 512 : (n + 1) * 512],
                                yt[:, k, :],
                                wt[s][:, n, k, :],
                                start=False,
                                stop=(k == kc - 1),
                                skip_group_check=True,
                            )

                def sq_then_cast(m, s):
                    """After stage s-1's matmuls: row sumsq on ScalarE."""
                    nc.scalar.activation(
                        out=sq_scr, in_=pr_of[m], func=square,
                        accum_out=new_ss(m, s)[:, :],
                    )

                def out_chain(m):
                    rs = rs_chain(m, 3, eps_t)
                    y3 = yhat_pool.tile(
                        [128, hidden], BF16, tag="yh", name=f"y3_{m}"
                    )
                    # y3 = (psum * rs) * g3 in one DVE op
                    nc.vector.scalar_tensor_tensor(
                        out=y3, in0=pr_of[m], scalar=rs[:, :], in1=g3t,
                        op0=mult, op1=mult,
                    )
                    nc.gpsimd.dma_start(
                        out=out_d[m * 128 : (m + 1) * 128, :], in_=y3
                    )

                # Pair-lockstep software pipeline: tiles (A, B) march
                # through stages together, so each tile's boundary chain
                # hides under the other tile's matmul block. The previous
                # pair's out-chains and this pair's seeds are interleaved
                # so they run in the shadow of the trailing matmul blocks.
                prev = None
                for a in range(0, nt_all, 2):
                    b = a + 1
                    if a + 2 < nt_all:
                        xload(a + 2)
                        xload(b + 2)
                    if prev is None:
                        ytA = seed_chain(a)
                        ytB = seed_chain(b)
                        mm(a, 0, ytA)
                        mm(b, 0, ytB)
                    else:
                        pa, pb = prev
                        sq_then_cast(pa, 3)
                        out_chain(pa)
                        ytA = seed_chain(a)
                        mm(a, 0, ytA)
                        sq_then_cast(pb, 3)
                        out_chain(pb)
                        ytB = seed_chain(b)
                        mm(b, 0, ytB)
                    for s in (1, 2):
                        sq_then_cast(a, s)
                        ytA = cast_t(a, s)
                        mm(a, s, ytA)
                        sq_then_cast(b, s)
                        ytB = cast_t(b, s)
                        mm(b, s, ytB)
                    prev = (a, b)
                pa, pb = prev
                sq_then_cast(pa, 3)
                out_chain(pa)
                sq_then_cast(pb, 3)
                out_chain(pb)

            if reps == 1:
                body()
            elif reps < 0:  # unrolled (sim analysis only)
                for _ in range(-reps):
                    body()
            else:
                with tc.For_i(0, reps, 1):
                    body()

    nc.compile()
    return nc


_CACHE = {}


def _get_program(key=(T_CORE, HIDDEN, TB)):  # noqa: B008
    if key not in _CACHE:
        _CACHE[key] = build_program(*key)
    return _CACHE[key]


def make_in_maps(inputs):
    """Host-side prep: fold g into W, scale by WS, cast, pre-tile, shard."""
    x = np.asarray(inputs["x"], dtype=np.float32).astype(ml_dtypes.bfloat16)
    kc, nb = HIDDEN // 128, HIDDEN // 512
    ws = []
    for i in range(3):
        w = np.asarray(inputs[f"W{i}"], dtype=np.float32)
        g = np.asarray(inputs[f"g{i}"], dtype=np.float32)
        w8 = (WS * g[:, None] * w).astype(ml_dtypes.float8_e3m4)
        w8 = w8.reshape(kc, 128, nb, 512).transpose(2, 1, 0, 3)
        ws.append(np.ascontiguousarray(w8.reshape(nb * 128, kc * 512)))
    g3 = np.asarray(inputs["g3"], dtype=np.float32).astype(ml_dtypes.bfloat16)

    in_maps = []
    for c in range(N_CORES):
        im = {"x": np.ascontiguousarray(x[c * T_CORE : (c + 1) * T_CORE])}
        for i in range(3):
            im[f"W{i}"] = ws[i]
        im["g3"] = g3
        in_maps.append(im)
    return in_maps


def run(inputs, trace=False):
    """Run on 8 NeuronCores. Returns (out, BassKernelResults)."""
    nc = _get_program()
    in_maps = make_in_maps(inputs)
    res = run_bass_kernel_spmd(nc, in_maps, list(range(N_CORES)), trace=trace)
    out = np.concatenate(
        [res.results[c]["out"].astype(np.float32) for c in range(N_CORES)], axis=0
    )
    return out, res


def kernel(**inputs) -> np.ndarray:
    out, _ = run(inputs, trace=False)
    return out
